# revision 14
# baseline (speedup 1.0000x reference)
"""Trainium2 Bass kernel for LuluAttention (gated GQA attention + RoPE).

Sharding over 8 NeuronCores: core = b*4 + g where b = batch (2), g = head
group (4). Each core computes 4 Q heads + their shared KV head for one batch
element, plus the matching gate slice, and a partial o_proj output
(contraction over its 512 attn dims). Host sums the 4 partials per batch.

On-chip layouts are transposed ([dim, seq]) so the attention pipeline needs
no transposes:
  qT/kT [d, s] -> scoresT[sk, sq] = kT_tile.T @ qT_chunk
  softmax denominator via ones-matmul (partition reduction), broadcast of the
  denominator via a K=1 bf16 matmul; the reciprocal is taken on the broadcast
  [128, 512] tile (partition-parallel) and fused with the sigmoid gate:
    ag = at / ((1 + exp(-z_gate)) * denom)
  v kept straight [s, d] -> attnT[d, sq] = v_tile.T @ probsT.

Perf structure:
  - All DRAM tensors are host-pre-arranged into their exact SBUF layouts so
    every DMA is contiguous per partition (128 big descriptors per load).
  - Causal narrowing: for diagonal k-tiles only columns sq >= o*128 are
    computed in scores/exp/AV/denominator; the remaining triangular mask is a
    single [128,128] multiply.
  - Scores are issued two k-tiles ahead of the AV matmuls so the scalar
    engine's exp latency is hidden behind PE work.
  - RoPE rotate-half (cross-partition move by 64) via DVE stream_shuffle;
    signs folded into the host-precomputed sin table.
"""

import numpy as np
import ml_dtypes
from contextlib import ExitStack

import concourse.bass as bass
import concourse.bacc as bacc
import concourse.tile as tile
from concourse import mybir
from concourse.bass_utils import run_bass_kernel_spmd

BF16 = ml_dtypes.bfloat16

HIDDEN = 2048
B = 2
S_FULL = 2048
P = 128
CH = 512               # seq chunk width
QH = 4                 # q heads per core
DQ = QH * P            # 512 q dims per core
KT = HIDDEN // P       # 16 contraction tiles
SCALE = 1.0 / float(np.sqrt(128.0))
ROPE_THETA = 10000.0

IDENT32 = list(range(32))


def build_program(S=S_FULL):
    f32 = mybir.dt.float32
    bf16 = mybir.dt.bfloat16
    expf = mybir.ActivationFunctionType.Exp

    NCH = S // CH
    ST = CH // P           # 4 seq sub-tiles per chunk

    nc = bacc.Bacc("TRN2", debug=False, target_bir_lowering=False)

    xT = nc.declare_dram_parameter("xT", [NCH, P, KT, CH], bf16, False)
    wq = nc.declare_dram_parameter("wq", [P, KT, DQ], bf16, False)
    wk = nc.declare_dram_parameter("wk", [P, KT, P], bf16, False)
    wv = nc.declare_dram_parameter("wv", [P, KT, P], bf16, False)
    wg = nc.declare_dram_parameter("wg", [P, KT, DQ], bf16, False)
    wo = nc.declare_dram_parameter("wo", [P, QH, HIDDEN], bf16, False)
    bgn = nc.declare_dram_parameter("bgn", [P, QH], f32, False)
    cosT = nc.declare_dram_parameter("cosT", [P, S], f32, False)
    sinT = nc.declare_dram_parameter("sinT", [P, S], f32, False)
    msk = nc.declare_dram_parameter("msk", [P, P], bf16, False)
    out = nc.declare_dram_parameter("out", [S, HIDDEN], f32, True)

    with tile.TileContext(nc) as tc, ExitStack() as ctx:
        wpool = ctx.enter_context(tc.tile_pool(name="weights", bufs=1))
        xpool = ctx.enter_context(tc.tile_pool(name="xchunks", bufs=2))
        qkv = ctx.enter_context(tc.tile_pool(name="qkv", bufs=1))
        egp = ctx.enter_context(tc.tile_pool(name="eg", bufs=2))
        work = ctx.enter_context(tc.tile_pool(name="work", bufs=2))
        prp = ctx.enter_context(tc.tile_pool(name="probs", bufs=4))
        nrm = ctx.enter_context(tc.tile_pool(name="nrm", bufs=2))
        agp = ctx.enter_context(tc.tile_pool(name="agp", bufs=2))
        outp = ctx.enter_context(tc.tile_pool(name="outp", bufs=2))
        ps_mm = ctx.enter_context(tc.tile_pool(name="ps_mm", bufs=2, space="PSUM"))
        ps_sc = ctx.enter_context(tc.tile_pool(name="ps_sc", bufs=2, space="PSUM"))
        ps_at = ctx.enter_context(tc.tile_pool(name="ps_at", bufs=2, space="PSUM"))
        ps_dn = ctx.enter_context(tc.tile_pool(name="ps_dn", bufs=1, space="PSUM"))

        # ---- persistent loads; x chunk 0 first so k/v proj start ASAP.
        # Early loads are split into multiple dma_starts so they spread
        # across DMA queues and the first matmul isn't issue-bound. ----
        xcs = [None] * NCH
        xcs[0] = xpool.tile([P, KT, CH], bf16, tag="xc", name="xc0")
        for k0 in range(0, KT, 4):
            nc.sync.dma_start(out=xcs[0][:, k0:k0 + 4, :],
                              in_=xT[0, :, k0:k0 + 4, :])
        wk_sb = wpool.tile([P, KT, P], bf16, tag="wk")
        for k0 in range(0, KT, 8):
            nc.sync.dma_start(out=wk_sb[:, k0:k0 + 8, :],
                              in_=wk[:, k0:k0 + 8, :])
        wv_sb = wpool.tile([P, KT, P], bf16, tag="wv")
        nc.sync.dma_start(out=wv_sb, in_=wv[:, :, :])
        cos_sb = wpool.tile([P, S], f32, tag="cos")
        nc.sync.dma_start(out=cos_sb, in_=cosT[:, :])
        sin_sb = wpool.tile([P, S], f32, tag="sin")
        nc.sync.dma_start(out=sin_sb, in_=sinT[:, :])
        wq_sb = wpool.tile([P, KT, DQ], bf16, tag="wq")
        for k0 in range(0, KT, 8):
            nc.sync.dma_start(out=wq_sb[:, k0:k0 + 8, :],
                              in_=wq[:, k0:k0 + 8, :])
        msk_sb = wpool.tile([P, P], bf16, tag="msk")
        nc.sync.dma_start(out=msk_sb, in_=msk[:, :])
        wg_sb = wpool.tile([P, KT, DQ], bf16, tag="wg")
        nc.sync.dma_start(out=wg_sb, in_=wg[:, :, :])
        bgn_sb = wpool.tile([P, QH], f32, tag="bgn")
        nc.sync.dma_start(out=bgn_sb, in_=bgn[:, :])
        wo_sb = wpool.tile([P, QH, HIDDEN], bf16, tag="wo")
        nc.sync.dma_start(out=wo_sb, in_=wo[:, :, :])
        ones_pv = wpool.tile([P, 1], bf16, tag="ones_pv")
        nc.vector.memset(ones_pv, 1.0)
        ones_bc = wpool.tile([1, P], bf16, tag="ones_bc")
        nc.vector.memset(ones_bc, 1.0)

        # persistent per-core activations (transposed layouts)
        qro = qkv.tile([P, QH, S], bf16, tag="qro")
        kro = qkv.tile([P, S], bf16, tag="kro")
        v_sb = qkv.tile([P, S // P, P], bf16, tag="v")

        def do_oproj(ci, ag_t):
            for st in range(ST):
                r0 = ci * CH + st * P
                obt = outp.tile([P, HIDDEN], f32, tag="obt")
                for hp in range(2):
                    pss = [
                        ps_mm.tile([P, CH], f32, tag="proj", name=f"ops{hi}")
                        for hi in range(2)
                    ]
                    for dt in range(QH):
                        for hi in range(2):
                            h0 = hp * 2 + hi
                            nc.tensor.matmul(
                                pss[hi],
                                ag_t[:, dt, st * P:(st + 1) * P],
                                wo_sb[:, dt, h0 * CH:(h0 + 1) * CH],
                                start=(dt == 0), stop=(dt == QH - 1),
                            )
                    for hi in range(2):
                        h0 = hp * 2 + hi
                        if hi == 0:
                            nc.vector.tensor_copy(
                                out=obt[:, h0 * CH:(h0 + 1) * CH], in_=pss[hi]
                            )
                        else:
                            nc.scalar.copy(
                                out=obt[:, h0 * CH:(h0 + 1) * CH], in_=pss[hi]
                            )
                nc.sync.dma_start(out=out[r0:r0 + P, :], in_=obt)

        # normalization fused with the sigmoid gate:
        # ag = at / ((1 + Eg) * denom), split so the bc matmul never waits
        # on the dnsb scalar copy at a head boundary.
        def norm_stage2(qh, at, dnsb, eg_t, ag_t):
            bc = ps_mm.tile([P, CH], f32, tag="proj", name="bc")
            nc.tensor.matmul(bc, ones_bc, dnsb, start=True, stop=True)
            w = nrm.tile([P, CH], f32, tag="w")
            nc.vector.scalar_tensor_tensor(
                out=w, in0=eg_t[:, qh, :], scalar=1.0, in1=bc,
                op0=mybir.AluOpType.add, op1=mybir.AluOpType.mult,
            )
            r = nrm.tile([P, CH], f32, tag="r")
            nc.vector.reciprocal_approx_fast(out=r, in_=w)
            nc.vector.tensor_mul(ag_t[:, qh, :], at, r)

        ag_prev = None
        pending_norm = None

        for c in range(NCH):
            cs = slice(c * CH, (c + 1) * CH)
            xc = xcs[c]

            def rope_head(ps, dst):
                qf = work.tile([P, CH], f32, tag="qf")
                nc.scalar.copy(out=qf, in_=ps)
                rot = work.tile([P, CH], f32, tag="rot")
                nc.vector.stream_shuffle(rot[0:64, :], qf[64:128, :], IDENT32)
                nc.vector.stream_shuffle(rot[64:128, :], qf[0:64, :], IDENT32)
                t1 = work.tile([P, CH], f32, tag="t1")
                nc.vector.tensor_mul(t1, qf, cos_sb[:, cs])
                t2 = work.tile([P, CH], f32, tag="t2")
                nc.vector.tensor_mul(t2, rot, sin_sb[:, cs])
                nc.vector.tensor_add(dst, t1, t2)

            # ---- k projection + RoPE ----
            psk = ps_mm.tile([P, CH], f32, tag="proj")
            for kt in range(KT):
                nc.tensor.matmul(
                    psk, wk_sb[:, kt, :], xc[:, kt, :],
                    start=(kt == 0), stop=(kt == KT - 1),
                )
            rope_head(psk, kro[:, cs])

            # ---- deferred norm tail of the previous chunk's last head
            # (its dnsb copy is long done; the remaining proj PE work hides
            # the bc/stt/recip/mul chain completely) ----
            if pending_norm is not None:
                norm_stage2(*pending_norm)
                pending_norm = None

            # ---- v projection (straight layout [s, d]) ----
            # 4 st-tiles land in disjoint 128-col regions of one PSUM bank
            psv = ps_mm.tile([P, CH], f32, tag="proj", name="psv")
            for st in range(ST):
                for kt in range(KT):
                    nc.tensor.matmul(
                        psv[:, st * P:(st + 1) * P],
                        xc[:, kt, st * P:(st + 1) * P], wv_sb[:, kt, :],
                        start=(kt == 0), stop=(kt == KT - 1),
                    )
            nc.scalar.copy(out=v_sb[:, c * ST:(c + 1) * ST, :], in_=psv)

            # ---- q heads + RoPE ----
            for qh in range(QH):
                psq = ps_mm.tile([P, CH], f32, tag="proj")
                for kt in range(KT):
                    nc.tensor.matmul(
                        psq, wq_sb[:, kt, qh * P:(qh + 1) * P], xc[:, kt, :],
                        start=(kt == 0), stop=(kt == KT - 1),
                    )
                rope_head(psq, qro[:, qh, cs])

            # ---- gate heads: Eg = exp(-(z + bg)); sigmoid folded into norm ----
            eg = egp.tile([P, QH, CH], bf16, tag="eg")
            for qh in range(QH):
                psg = ps_mm.tile([P, CH], f32, tag="proj")
                for kt in range(KT):
                    nc.tensor.matmul(
                        psg, wg_sb[:, kt, qh * P:(qh + 1) * P], xc[:, kt, :],
                        start=(kt == 0), stop=(kt == KT - 1),
                    )
                nc.scalar.activation(
                    out=eg[:, qh, :], in_=psg, func=expf,
                    bias=bgn_sb[:, qh:qh + 1], scale=-1.0,
                )

            # prefetch next x chunk while attention runs
            if c + 1 < NCH:
                xcs[c + 1] = xpool.tile([P, KT, CH], bf16, tag="xc",
                                        name=f"xc{c + 1}")
                nc.sync.dma_start(out=xcs[c + 1], in_=xT[c + 1, :, :, :])

            # ---- deferred o_proj of the PREVIOUS chunk (its ag is long
            # ready; keeps the norm chain off the PE critical path) ----
            if c > 0:
                do_oproj(c - 1, ag_prev)

            # ---- attention for this sq chunk ----
            # One flat (qh, t) stream: the score-issue pointer runs two
            # tiles ahead ACROSS head boundaries, so the PE never waits for
            # exp latency when switching heads.
            ag = agp.tile([P, QH, CH], bf16, tag="ag")
            ntiles = (c + 1) * ST
            items = [(qh, t) for qh in range(QH) for t in range(ntiles)]
            sc_slots = {}

            def issue_sc(qh, t):
                o = t - c * ST
                lo = o * P if o > 0 else 0
                s = ps_sc.tile([P, CH], f32, tag="sc")
                nc.tensor.matmul(
                    s[:, lo:], kro[:, t * P:(t + 1) * P],
                    qro[:, qh, c * CH + lo:(c + 1) * CH],
                    start=True, stop=True,
                )
                sc_slots[(qh, t)] = (s, lo)

            issue_sc(*items[0])
            if len(items) > 1:
                issue_sc(*items[1])
            at = dn = None
            for i, (qh, t) in enumerate(items):
                if t == 0:
                    at = ps_at.tile([P, CH], f32, tag="attn")
                    dn = ps_dn.tile([1, CH], f32, tag="denom")
                s, lo = sc_slots.pop((qh, t))
                o = t - c * ST
                pr = prp.tile([P, CH], bf16, tag="pr")
                nc.scalar.activation(
                    out=pr[:, lo:], in_=s[:, lo:], func=expf, scale=SCALE,
                )
                if o >= 0:
                    nc.vector.tensor_mul(
                        pr[:, o * P:(o + 1) * P],
                        pr[:, o * P:(o + 1) * P], msk_sb,
                    )
                nc.tensor.matmul(
                    at[:, lo:], v_sb[:, t, :], pr[:, lo:],
                    start=(t == 0), stop=(t == ntiles - 1),
                )
                nc.tensor.matmul(
                    dn[:, lo:], ones_pv, pr[:, lo:],
                    start=(t == 0), stop=(t == ntiles - 1),
                )
                if i + 2 < len(items):
                    issue_sc(*items[i + 2])
                if t == 1 and pending_norm is not None:
                    norm_stage2(*pending_norm)
                    pending_norm = None
                if t == ntiles - 1:
                    # stage 1 on the vector engine: frees the single dn bank
                    # early without wedging a copy into the scalar exp stream
                    dnsb = nrm.tile([1, CH], bf16, tag="dnsb")
                    nc.vector.tensor_copy(out=dnsb, in_=dn)
                    pending_norm = (qh, at, dnsb, eg, ag)
            ag_prev = ag

        # final chunk tail: last head's norm, then o_proj
        norm_stage2(*pending_norm)
        do_oproj(NCH - 1, ag_prev)

    nc.finalize()
    return nc


_PROGRAMS = {}


def _get_program(S=S_FULL):
    if S not in _PROGRAMS:
        _PROGRAMS[S] = build_program(S)
    return _PROGRAMS[S]


def _host_tables(position_ids_b, S):
    pos = np.asarray(position_ids_b, dtype=np.float32)  # [S]
    inv = 1.0 / (ROPE_THETA ** (np.arange(0, P, 2, dtype=np.float32) / P))  # [64]
    ang = np.concatenate([inv, inv]).astype(np.float32)[:, None] * pos[None, :]
    cosT = np.cos(ang).astype(np.float32)
    sgn = np.where(np.arange(P) < 64, -1.0, 1.0).astype(np.float32)
    sinT = (np.sin(ang) * sgn[:, None]).astype(np.float32)
    return cosT, sinT


def make_in_maps(x, position_ids, Wq, Wk, Wv, Wo, Wg, bg, S=S_FULL):
    NCH = S // CH
    x = np.asarray(x, dtype=np.float32)
    msk = (np.arange(P)[:, None] <= np.arange(P)[None, :]).astype(BF16)
    xT_b = []
    for b in range(B):
        xt = np.ascontiguousarray(x[b, :S].T)                    # [H, S]
        xt = xt.reshape(KT, P, NCH, CH).transpose(2, 1, 0, 3)    # [NCH,P,KT,CH]
        xT_b.append(np.ascontiguousarray(xt).astype(BF16))
    tabs = [_host_tables(np.asarray(position_ids)[b, :S], S) for b in range(B)]
    Wq = np.asarray(Wq, np.float32)
    Wk = np.asarray(Wk, np.float32)
    Wv = np.asarray(Wv, np.float32)
    Wo = np.asarray(Wo, np.float32)
    Wg = np.asarray(Wg, np.float32)
    bg = np.asarray(bg, np.float32)

    def warr(w):  # [H, N] -> [P, KT, N]
        n = w.shape[1]
        return np.ascontiguousarray(
            w.reshape(KT, P, n).transpose(1, 0, 2)).astype(BF16)

    maps = []
    for core in range(8):
        b, g = core // 4, core % 4
        cosT, sinT = tabs[b]
        wo_c = Wo[g * DQ:(g + 1) * DQ, :].reshape(QH, P, HIDDEN).transpose(1, 0, 2)
        bgn_c = (-bg[g * DQ:(g + 1) * DQ]).reshape(QH, P).T
        maps.append({
            "xT": xT_b[b],
            "wq": warr(Wq[:, g * DQ:(g + 1) * DQ]),
            "wk": warr(Wk[:, g * P:(g + 1) * P]),
            "wv": warr(Wv[:, g * P:(g + 1) * P]),
            "wg": warr(Wg[:, g * DQ:(g + 1) * DQ]),
            "wo": np.ascontiguousarray(wo_c).astype(BF16),
            "bgn": np.ascontiguousarray(bgn_c),
            "cosT": cosT,
            "sinT": sinT,
            "msk": msk,
        })
    return maps


def run(inputs, S=S_FULL, trace=False, **kw):
    nc = _get_program(S)
    maps = make_in_maps(S=S, **inputs)
    res = run_bass_kernel_spmd(nc, maps, core_ids=list(range(8)), trace=trace, **kw)
    out = np.zeros((B, S, HIDDEN), np.float32)
    for core in range(8):
        out[core // 4] += np.asarray(res.results[core]["out"], np.float32)
    return out, res


def kernel(x, position_ids, Wq, Wk, Wv, Wo, Wg, bg):
    out, _ = run(dict(x=x, position_ids=position_ids, Wq=Wq, Wk=Wk, Wv=Wv,
                      Wo=Wo, Wg=Wg, bg=bg))
    return out


# revision 17
# speedup vs baseline: 1.2620x; 1.2620x over previous
"""Trainium2 Bass kernel for LuluAttention (gated GQA attention + RoPE).

Sharding over 8 NeuronCores: core = b*4 + g where b = batch (2), g = head
group (4). Each core computes 4 Q heads + their shared KV head for one batch
element, plus the matching gate slice, and a partial o_proj output
(contraction over its 512 attn dims). Host sums the 4 partials per batch.

On-chip layouts are transposed ([dim, seq]) so the attention pipeline needs
no transposes:
  qT/kT [d, s] -> scoresT[sk, sq] = kT_tile.T @ qT_chunk
  softmax denominator via ones-matmul (partition reduction), broadcast of the
  denominator via a K=1 bf16 matmul; the reciprocal is taken on the broadcast
  [128, 512] tile (partition-parallel) and fused with the sigmoid gate:
    ag = at / ((1 + exp(-z_gate)) * denom)
  v kept straight [s, d] -> attnT[d, sq] = v_tile.T @ probsT.

Perf structure:
  - All DRAM tensors are host-pre-arranged into their exact SBUF layouts so
    every DMA is contiguous per partition (128 big descriptors per load).
  - Causal narrowing: for diagonal k-tiles only columns sq >= o*128 are
    computed in scores/exp/AV/denominator; the remaining triangular mask is a
    single [128,128] multiply.
  - Scores are issued two k-tiles ahead of the AV matmuls so the scalar
    engine's exp latency is hidden behind PE work.
  - RoPE rotate-half (cross-partition move by 64) via DVE stream_shuffle;
    signs folded into the host-precomputed sin table.
"""

import numpy as np
import ml_dtypes
from contextlib import ExitStack

import concourse.bass as bass
import concourse.bacc as bacc
import concourse.tile as tile
from concourse import mybir
from concourse.bass_utils import run_bass_kernel_spmd

BF16 = ml_dtypes.bfloat16

HIDDEN = 2048
B = 2
S_FULL = 2048
P = 128
CH = 512               # seq chunk width
QH = 4                 # q heads per core
DQ = QH * P            # 512 q dims per core
KT = HIDDEN // P       # 16 contraction tiles
SCALE = 1.0 / float(np.sqrt(128.0))
ROPE_THETA = 10000.0

IDENT32 = list(range(32))


def build_program(S=S_FULL):
    f32 = mybir.dt.float32
    bf16 = mybir.dt.bfloat16
    f8 = mybir.dt.float8e4
    expf = mybir.ActivationFunctionType.Exp
    DROW = mybir.MatmulPerfMode.DoubleRow

    NCH = S // CH
    ST = CH // P           # 4 seq sub-tiles per chunk

    nc = bacc.Bacc("TRN2", debug=False, target_bir_lowering=False)

    xT = nc.declare_dram_parameter("xT", [NCH, P, KT, CH], bf16, False)
    wq = nc.declare_dram_parameter("wq", [P, KT, DQ], bf16, False)
    wk = nc.declare_dram_parameter("wk", [P, KT, P], bf16, False)
    wv = nc.declare_dram_parameter("wv", [P, KT, P], bf16, False)
    wg = nc.declare_dram_parameter("wg", [P, KT, DQ], bf16, False)
    wo = nc.declare_dram_parameter("wo", [P, QH, HIDDEN], bf16, False)
    bgn = nc.declare_dram_parameter("bgn", [P, QH], f32, False)
    cosT = nc.declare_dram_parameter("cosT", [P, S], f32, False)
    sinT = nc.declare_dram_parameter("sinT", [P, S], f32, False)
    msk = nc.declare_dram_parameter("msk", [P, P], bf16, False)
    out = nc.declare_dram_parameter("out", [S, HIDDEN], f32, True)

    with tile.TileContext(nc) as tc, ExitStack() as ctx:
        wpool = ctx.enter_context(tc.tile_pool(name="weights", bufs=1))
        xpool = ctx.enter_context(tc.tile_pool(name="xchunks", bufs=2))
        qkv = ctx.enter_context(tc.tile_pool(name="qkv", bufs=1))
        egp = ctx.enter_context(tc.tile_pool(name="eg", bufs=2))
        work = ctx.enter_context(tc.tile_pool(name="work", bufs=2))
        prp = ctx.enter_context(tc.tile_pool(name="probs", bufs=4))
        nrm = ctx.enter_context(tc.tile_pool(name="nrm", bufs=2))
        agp = ctx.enter_context(tc.tile_pool(name="agp", bufs=2))
        outp = ctx.enter_context(tc.tile_pool(name="outp", bufs=2))
        ps_mm = ctx.enter_context(tc.tile_pool(name="ps_mm", bufs=2, space="PSUM"))
        ps_sc = ctx.enter_context(tc.tile_pool(name="ps_sc", bufs=2, space="PSUM"))
        ps_at = ctx.enter_context(tc.tile_pool(name="ps_at", bufs=2, space="PSUM"))
        ps_dn = ctx.enter_context(tc.tile_pool(name="ps_dn", bufs=1, space="PSUM"))

        # ---- persistent loads; x chunk 0 first so k/v proj start ASAP.
        # Early loads are split into multiple dma_starts so they spread
        # across DMA queues and the first matmul isn't issue-bound. ----
        xcs = [None] * NCH
        xcs[0] = xpool.tile([P, KT, CH], bf16, tag="xc", name="xc0")
        for k0 in range(0, KT, 4):
            nc.sync.dma_start(out=xcs[0][:, k0:k0 + 4, :],
                              in_=xT[0, :, k0:k0 + 4, :])
        wk_sb = wpool.tile([P, KT, P], bf16, tag="wk")
        for k0 in range(0, KT, 8):
            nc.sync.dma_start(out=wk_sb[:, k0:k0 + 8, :],
                              in_=wk[:, k0:k0 + 8, :])
        wv_sb = wpool.tile([P, KT, P], bf16, tag="wv")
        nc.sync.dma_start(out=wv_sb, in_=wv[:, :, :])
        cos_sb = wpool.tile([P, S], f32, tag="cos")
        nc.sync.dma_start(out=cos_sb, in_=cosT[:, :])
        sin_sb = wpool.tile([P, S], f32, tag="sin")
        nc.sync.dma_start(out=sin_sb, in_=sinT[:, :])
        wq_sb = wpool.tile([P, KT, DQ], bf16, tag="wq")
        for k0 in range(0, KT, 8):
            nc.sync.dma_start(out=wq_sb[:, k0:k0 + 8, :],
                              in_=wq[:, k0:k0 + 8, :])
        msk_sb = wpool.tile([P, P], bf16, tag="msk")
        nc.sync.dma_start(out=msk_sb, in_=msk[:, :])
        wg_sb = wpool.tile([P, KT, DQ], bf16, tag="wg")
        nc.sync.dma_start(out=wg_sb, in_=wg[:, :, :])
        bgn_sb = wpool.tile([P, QH], f32, tag="bgn")
        nc.sync.dma_start(out=bgn_sb, in_=bgn[:, :])
        wo_sb = wpool.tile([P, QH, HIDDEN], bf16, tag="wo")
        nc.sync.dma_start(out=wo_sb, in_=wo[:, :, :])
        ones_pv = wpool.tile([P, 1], bf16, tag="ones_pv")
        nc.vector.memset(ones_pv, 1.0)
        ones_bc = wpool.tile([1, P], bf16, tag="ones_bc")
        nc.vector.memset(ones_bc, 1.0)
        # fp8 ones for the DoubleRow denominator; the 2-dim stride is kept
        # 16B-aligned by the [P, 2, 16] layout
        ones_f8 = wpool.tile([P, 2, 16], f8, tag="ones_f8")
        nc.vector.memset(ones_f8, 1.0)

        # persistent per-core activations (transposed layouts)
        qro = qkv.tile([P, QH, S], bf16, tag="qro")
        kro = qkv.tile([P, S], bf16, tag="kro")
        v_sb = qkv.tile([P, S // P, P], bf16, tag="v")

        def do_oproj(ci, ag_t):
            for st in range(ST):
                r0 = ci * CH + st * P
                obt = outp.tile([P, HIDDEN], f32, tag="obt")
                for hp in range(2):
                    pss = [
                        ps_mm.tile([P, CH], f32, tag="proj", name=f"ops{hi}")
                        for hi in range(2)
                    ]
                    for dt in range(QH):
                        for hi in range(2):
                            h0 = hp * 2 + hi
                            nc.tensor.matmul(
                                pss[hi],
                                ag_t[:, dt, st * P:(st + 1) * P],
                                wo_sb[:, dt, h0 * CH:(h0 + 1) * CH],
                                start=(dt == 0), stop=(dt == QH - 1),
                            )
                    for hi in range(2):
                        h0 = hp * 2 + hi
                        if hi == 0:
                            nc.vector.tensor_copy(
                                out=obt[:, h0 * CH:(h0 + 1) * CH], in_=pss[hi]
                            )
                        else:
                            nc.scalar.copy(
                                out=obt[:, h0 * CH:(h0 + 1) * CH], in_=pss[hi]
                            )
                nc.sync.dma_start(out=out[r0:r0 + P, :], in_=obt)

        # normalization fused with the sigmoid gate:
        # ag = at / ((1 + Eg) * denom), split so the bc matmul never waits
        # on the dnsb scalar copy at a head boundary.
        def norm_stage2(qh, at, dnsb, eg_t, ag_t):
            bc = ps_mm.tile([P, CH], f32, tag="proj", name="bc")
            nc.tensor.matmul(bc, ones_bc, dnsb, start=True, stop=True)
            w = nrm.tile([P, CH], f32, tag="w")
            nc.vector.scalar_tensor_tensor(
                out=w, in0=eg_t[:, qh, :], scalar=1.0, in1=bc,
                op0=mybir.AluOpType.add, op1=mybir.AluOpType.mult,
            )
            r = nrm.tile([P, CH], f32, tag="r")
            nc.vector.reciprocal_approx_fast(out=r, in_=w)
            nc.vector.tensor_mul(ag_t[:, qh, :], at, r)

        ag_prev = None
        pending_norm = None

        for c in range(NCH):
            cs = slice(c * CH, (c + 1) * CH)
            xc = xcs[c]

            def rope_head(ps, dst):
                qf = work.tile([P, CH], f32, tag="qf")
                nc.scalar.copy(out=qf, in_=ps)
                rot = work.tile([P, CH], f32, tag="rot")
                nc.vector.stream_shuffle(rot[0:64, :], qf[64:128, :], IDENT32)
                nc.vector.stream_shuffle(rot[64:128, :], qf[0:64, :], IDENT32)
                t1 = work.tile([P, CH], f32, tag="t1")
                nc.vector.tensor_mul(t1, qf, cos_sb[:, cs])
                t2 = work.tile([P, CH], f32, tag="t2")
                nc.vector.tensor_mul(t2, rot, sin_sb[:, cs])
                nc.vector.tensor_add(dst, t1, t2)

            # ---- k projection + RoPE ----
            psk = ps_mm.tile([P, CH], f32, tag="proj")
            for kt in range(KT):
                nc.tensor.matmul(
                    psk, wk_sb[:, kt, :], xc[:, kt, :],
                    start=(kt == 0), stop=(kt == KT - 1),
                )
            rope_head(psk, kro[:, cs])

            # ---- deferred norm tail of the previous chunk's last head
            # (its dnsb copy is long done; the remaining proj PE work hides
            # the bc/stt/recip/mul chain completely) ----
            if pending_norm is not None:
                norm_stage2(*pending_norm)
                pending_norm = None

            # ---- v projection (straight layout [s, d]) ----
            # 4 st-tiles land in disjoint 128-col regions of one PSUM bank
            psv = ps_mm.tile([P, CH], f32, tag="proj", name="psv")
            for st in range(ST):
                for kt in range(KT):
                    nc.tensor.matmul(
                        psv[:, st * P:(st + 1) * P],
                        xc[:, kt, st * P:(st + 1) * P], wv_sb[:, kt, :],
                        start=(kt == 0), stop=(kt == KT - 1),
                    )
            nc.scalar.copy(out=v_sb[:, c * ST:(c + 1) * ST, :], in_=psv)

            # ---- q heads + RoPE ----
            for qh in range(QH):
                psq = ps_mm.tile([P, CH], f32, tag="proj")
                for kt in range(KT):
                    nc.tensor.matmul(
                        psq, wq_sb[:, kt, qh * P:(qh + 1) * P], xc[:, kt, :],
                        start=(kt == 0), stop=(kt == KT - 1),
                    )
                rope_head(psq, qro[:, qh, cs])

            # ---- gate heads: Eg = exp(-(z + bg)); sigmoid folded into norm ----
            eg = egp.tile([P, QH, CH], bf16, tag="eg")
            for qh in range(QH):
                psg = ps_mm.tile([P, CH], f32, tag="proj")
                for kt in range(KT):
                    nc.tensor.matmul(
                        psg, wg_sb[:, kt, qh * P:(qh + 1) * P], xc[:, kt, :],
                        start=(kt == 0), stop=(kt == KT - 1),
                    )
                nc.scalar.activation(
                    out=eg[:, qh, :], in_=psg, func=expf,
                    bias=bgn_sb[:, qh:qh + 1], scale=-1.0,
                )

            # prefetch next x chunk while attention runs
            if c + 1 < NCH:
                xcs[c + 1] = xpool.tile([P, KT, CH], bf16, tag="xc",
                                        name=f"xc{c + 1}")
                nc.sync.dma_start(out=xcs[c + 1], in_=xT[c + 1, :, :, :])

            # ---- deferred o_proj of the PREVIOUS chunk (its ag is long
            # ready; keeps the norm chain off the PE critical path) ----
            if c > 0:
                do_oproj(c - 1, ag_prev)

            # ---- attention for this sq chunk ----
            # One flat (qh, t) stream: the score-issue pointer runs two
            # tiles ahead ACROSS head boundaries, so the PE never waits for
            # exp latency when switching heads.
            ag = agp.tile([P, QH, CH], bf16, tag="ag")
            ntiles = (c + 1) * ST
            items = [(qh, t) for qh in range(QH) for t in range(ntiles)]
            sc_slots = {}

            def issue_sc(qh, t):
                o = t - c * ST
                lo = o * P if o > 0 else 0
                s = ps_sc.tile([P, CH], f32, tag="sc")
                nc.tensor.matmul(
                    s[:, lo:], kro[:, t * P:(t + 1) * P],
                    qro[:, qh, c * CH + lo:(c + 1) * CH],
                    start=True, stop=True,
                )
                sc_slots[(qh, t)] = (s, lo)

            issue_sc(*items[0])
            if len(items) > 1:
                issue_sc(*items[1])
            at = dn = pr8 = None
            for i, (qh, t) in enumerate(items):
                if t == 0:
                    at = ps_at.tile([P, CH], f32, tag="attn")
                    dn = ps_dn.tile([1, CH], f32, tag="denom")
                s, lo = sc_slots.pop((qh, t))
                o = t - c * ST
                if o < 0:
                    # full (non-diagonal) tile: probs in fp8, paired so the
                    # denominator runs as a DoubleRow matmul at 0.5 cyc/row
                    j = t % 2
                    if j == 0:
                        pr8 = prp.tile([P, 2, CH], f8, tag="pr8")
                    nc.scalar.activation(
                        out=pr8[:, j, :], in_=s, func=expf, scale=SCALE,
                    )
                    nc.tensor.matmul(
                        at, v_sb[:, t, :], pr8[:, j, :],
                        start=(t == 0), stop=False,
                    )
                    if j == 1:
                        nc.tensor.matmul(
                            dn, ones_f8[:, :, 0:1], pr8,
                            start=(t == 1), stop=False,
                            perf_mode=DROW,
                        )
                else:
                    pr = prp.tile([P, CH], bf16, tag="pr")
                    nc.scalar.activation(
                        out=pr[:, lo:], in_=s[:, lo:], func=expf, scale=SCALE,
                    )
                    nc.vector.tensor_mul(
                        pr[:, o * P:(o + 1) * P],
                        pr[:, o * P:(o + 1) * P], msk_sb,
                    )
                    nc.tensor.matmul(
                        at[:, lo:], v_sb[:, t, :], pr[:, lo:],
                        start=(t == 0), stop=(t == ntiles - 1),
                    )
                    nc.tensor.matmul(
                        dn[:, lo:], ones_pv, pr[:, lo:],
                        start=(c == 0 and t == 0), stop=(t == ntiles - 1),
                    )
                if i + 2 < len(items):
                    issue_sc(*items[i + 2])
                if t == 1 and pending_norm is not None:
                    norm_stage2(*pending_norm)
                    pending_norm = None
                if t == ntiles - 1:
                    # stage 1 on the vector engine: frees the single dn bank
                    # early without wedging a copy into the scalar exp stream
                    dnsb = nrm.tile([1, CH], bf16, tag="dnsb")
                    nc.vector.tensor_copy(out=dnsb, in_=dn)
                    pending_norm = (qh, at, dnsb, eg, ag)
            ag_prev = ag

        # final chunk tail: last head's norm, then o_proj
        norm_stage2(*pending_norm)
        do_oproj(NCH - 1, ag_prev)

    nc.finalize()
    return nc


_PROGRAMS = {}


def _get_program(S=S_FULL):
    if S not in _PROGRAMS:
        _PROGRAMS[S] = build_program(S)
    return _PROGRAMS[S]


def _host_tables(position_ids_b, S):
    pos = np.asarray(position_ids_b, dtype=np.float32)  # [S]
    inv = 1.0 / (ROPE_THETA ** (np.arange(0, P, 2, dtype=np.float32) / P))  # [64]
    ang = np.concatenate([inv, inv]).astype(np.float32)[:, None] * pos[None, :]
    cosT = np.cos(ang).astype(np.float32)
    sgn = np.where(np.arange(P) < 64, -1.0, 1.0).astype(np.float32)
    sinT = (np.sin(ang) * sgn[:, None]).astype(np.float32)
    return cosT, sinT


def make_in_maps(x, position_ids, Wq, Wk, Wv, Wo, Wg, bg, S=S_FULL):
    NCH = S // CH
    x = np.asarray(x, dtype=np.float32)
    msk = (np.arange(P)[:, None] <= np.arange(P)[None, :]).astype(BF16)
    xT_b = []
    for b in range(B):
        xt = np.ascontiguousarray(x[b, :S].T)                    # [H, S]
        xt = xt.reshape(KT, P, NCH, CH).transpose(2, 1, 0, 3)    # [NCH,P,KT,CH]
        xT_b.append(np.ascontiguousarray(xt).astype(BF16))
    tabs = [_host_tables(np.asarray(position_ids)[b, :S], S) for b in range(B)]
    Wq = np.asarray(Wq, np.float32)
    Wk = np.asarray(Wk, np.float32)
    Wv = np.asarray(Wv, np.float32)
    Wo = np.asarray(Wo, np.float32)
    Wg = np.asarray(Wg, np.float32)
    bg = np.asarray(bg, np.float32)

    def warr(w):  # [H, N] -> [P, KT, N]
        n = w.shape[1]
        return np.ascontiguousarray(
            w.reshape(KT, P, n).transpose(1, 0, 2)).astype(BF16)

    maps = []
    for core in range(8):
        b, g = core // 4, core % 4
        cosT, sinT = tabs[b]
        wo_c = Wo[g * DQ:(g + 1) * DQ, :].reshape(QH, P, HIDDEN).transpose(1, 0, 2)
        bgn_c = (-bg[g * DQ:(g + 1) * DQ]).reshape(QH, P).T
        maps.append({
            "xT": xT_b[b],
            "wq": warr(Wq[:, g * DQ:(g + 1) * DQ]),
            "wk": warr(Wk[:, g * P:(g + 1) * P]),
            "wv": warr(Wv[:, g * P:(g + 1) * P]),
            "wg": warr(Wg[:, g * DQ:(g + 1) * DQ]),
            "wo": np.ascontiguousarray(wo_c).astype(BF16),
            "bgn": np.ascontiguousarray(bgn_c),
            "cosT": cosT,
            "sinT": sinT,
            "msk": msk,
        })
    return maps


def run(inputs, S=S_FULL, trace=False, **kw):
    nc = _get_program(S)
    maps = make_in_maps(S=S, **inputs)
    res = run_bass_kernel_spmd(nc, maps, core_ids=list(range(8)), trace=trace, **kw)
    out = np.zeros((B, S, HIDDEN), np.float32)
    for core in range(8):
        out[core // 4] += np.asarray(res.results[core]["out"], np.float32)
    return out, res


def kernel(x, position_ids, Wq, Wk, Wv, Wo, Wg, bg):
    out, _ = run(dict(x=x, position_ids=position_ids, Wq=Wq, Wk=Wk, Wv=Wv,
                      Wo=Wo, Wg=Wg, bg=bg))
    return out


# revision 19
# speedup vs baseline: 1.3224x; 1.0479x over previous
"""Trainium2 Bass kernel for LuluAttention (gated GQA attention + RoPE).

Sharding over 8 NeuronCores: core = b*4 + g where b = batch (2), g = head
group (4). Each core computes 4 Q heads + their shared KV head for one batch
element, plus the matching gate slice, and a partial o_proj output
(contraction over its 512 attn dims). Host sums the 4 partials per batch.

On-chip layouts are transposed ([dim, seq]) so the attention pipeline needs
no transposes:
  qT/kT [d, s] -> scoresT[sk, sq] = kT_tile.T @ qT_chunk
  softmax denominator via ones-matmul (partition reduction); full tiles use
  fp8 probs in pairs so the denominator runs as a DoubleRow matmul at
  0.5 cyc/row.  The denominator broadcast is a K=1 bf16 matmul; the
  reciprocal (reciprocal_approx_fast) is fused with the sigmoid gate:
    ag = at / ((1 + exp(-z_gate)) * denom)
  v kept straight [s, d] -> attnT[d, sq] = v_tile.T @ probsT.

Perf structure:
  - All DRAM tensors host-pre-arranged into SBUF layouts (contiguous
    per-partition DMA descriptors).
  - Causal narrowing on diagonal tiles + single [128,128] triangle mask.
  - One flat (head, tile) attention stream per chunk; scores issued two
    tiles ahead across head boundaries.
  - Filler queue: the NEXT chunk's projections and the PREVIOUS chunk's
    o_proj are emitted at attention head boundaries, so the PE fills the
    exp-latency slack and never idles; attention keeps the scalar engine's
    queue pure exp (all PSUM-drain copies run on the vector engine).
  - Per-chunk activation tiles (kro/qro/v/eg) avoid false write-read
    dependencies between interleaved chunks.
"""

import numpy as np
import ml_dtypes
from contextlib import ExitStack

import concourse.bass as bass
import concourse.bacc as bacc
import concourse.tile as tile
from concourse import mybir
from concourse.bass_utils import run_bass_kernel_spmd

BF16 = ml_dtypes.bfloat16

HIDDEN = 2048
B = 2
S_FULL = 2048
P = 128
CH = 512               # seq chunk width
QH = 4                 # q heads per core
DQ = QH * P            # 512 q dims per core
KT = HIDDEN // P       # 16 contraction tiles
SCALE = 1.0 / float(np.sqrt(128.0))
ROPE_THETA = 10000.0

IDENT32 = list(range(32))


def build_program(S=S_FULL):
    f32 = mybir.dt.float32
    bf16 = mybir.dt.bfloat16
    f8 = mybir.dt.float8e4
    expf = mybir.ActivationFunctionType.Exp
    DROW = mybir.MatmulPerfMode.DoubleRow

    NCH = S // CH
    ST = CH // P           # 4 seq sub-tiles per chunk

    nc = bacc.Bacc("TRN2", debug=False, target_bir_lowering=False)

    xT = nc.declare_dram_parameter("xT", [NCH, P, KT, CH], bf16, False)
    wq = nc.declare_dram_parameter("wq", [P, KT, DQ], bf16, False)
    wk = nc.declare_dram_parameter("wk", [P, KT, P], bf16, False)
    wv = nc.declare_dram_parameter("wv", [P, KT, P], bf16, False)
    wg = nc.declare_dram_parameter("wg", [P, KT, DQ], bf16, False)
    wo = nc.declare_dram_parameter("wo", [P, QH, HIDDEN], bf16, False)
    bgn = nc.declare_dram_parameter("bgn", [P, QH], f32, False)
    cosT = nc.declare_dram_parameter("cosT", [P, S], f32, False)
    sinT = nc.declare_dram_parameter("sinT", [P, S], f32, False)
    msk = nc.declare_dram_parameter("msk", [P, P], bf16, False)
    out = nc.declare_dram_parameter("out", [S, HIDDEN], f32, True)

    with tile.TileContext(nc) as tc, ExitStack() as ctx:
        wpool = ctx.enter_context(tc.tile_pool(name="weights", bufs=1))
        xpool = ctx.enter_context(tc.tile_pool(name="xchunks", bufs=2))
        qkv = ctx.enter_context(tc.tile_pool(name="qkv", bufs=1))
        qrop = ctx.enter_context(tc.tile_pool(name="qrop", bufs=2))
        egp = ctx.enter_context(tc.tile_pool(name="eg", bufs=2))
        work = ctx.enter_context(tc.tile_pool(name="work", bufs=2))
        prp = ctx.enter_context(tc.tile_pool(name="probs", bufs=4))
        nrm = ctx.enter_context(tc.tile_pool(name="nrm", bufs=2))
        agp = ctx.enter_context(tc.tile_pool(name="agp", bufs=2))
        outp = ctx.enter_context(tc.tile_pool(name="outp", bufs=2))
        ps_mm = ctx.enter_context(tc.tile_pool(name="ps_mm", bufs=2, space="PSUM"))
        ps_sc = ctx.enter_context(tc.tile_pool(name="ps_sc", bufs=2, space="PSUM"))
        ps_at = ctx.enter_context(tc.tile_pool(name="ps_at", bufs=2, space="PSUM"))
        ps_dn = ctx.enter_context(tc.tile_pool(name="ps_dn", bufs=1, space="PSUM"))

        # ---- persistent loads; x chunk 0 first so k/v proj start ASAP.
        # Early loads split into several dma_starts to spread across queues.
        xcs = [None] * NCH
        xcs[0] = xpool.tile([P, KT, CH], bf16, tag="xc", name="xc0")
        for k0 in range(0, KT, 4):
            nc.sync.dma_start(out=xcs[0][:, k0:k0 + 4, :],
                              in_=xT[0, :, k0:k0 + 4, :])
        wk_sb = wpool.tile([P, KT, P], bf16, tag="wk")
        for k0 in range(0, KT, 8):
            nc.sync.dma_start(out=wk_sb[:, k0:k0 + 8, :],
                              in_=wk[:, k0:k0 + 8, :])
        wv_sb = wpool.tile([P, KT, P], bf16, tag="wv")
        nc.sync.dma_start(out=wv_sb, in_=wv[:, :, :])
        cos_sb = wpool.tile([P, S], f32, tag="cos")
        nc.sync.dma_start(out=cos_sb, in_=cosT[:, :])
        sin_sb = wpool.tile([P, S], f32, tag="sin")
        nc.sync.dma_start(out=sin_sb, in_=sinT[:, :])
        wq_sb = wpool.tile([P, KT, DQ], bf16, tag="wq")
        for k0 in range(0, KT, 8):
            nc.sync.dma_start(out=wq_sb[:, k0:k0 + 8, :],
                              in_=wq[:, k0:k0 + 8, :])
        msk_sb = wpool.tile([P, P], bf16, tag="msk")
        nc.sync.dma_start(out=msk_sb, in_=msk[:, :])
        wg_sb = wpool.tile([P, KT, DQ], bf16, tag="wg")
        nc.sync.dma_start(out=wg_sb, in_=wg[:, :, :])
        bgn_sb = wpool.tile([P, QH], f32, tag="bgn")
        nc.sync.dma_start(out=bgn_sb, in_=bgn[:, :])
        wo_sb = wpool.tile([P, QH, HIDDEN], bf16, tag="wo")
        nc.sync.dma_start(out=wo_sb, in_=wo[:, :, :])
        ones_pv = wpool.tile([P, 1], bf16, tag="ones_pv")
        nc.vector.memset(ones_pv, 1.0)
        ones_bc = wpool.tile([1, P], bf16, tag="ones_bc")
        nc.vector.memset(ones_bc, 1.0)
        # fp8 ones for the DoubleRow denominator; 2-dim stride kept 16B-aligned
        ones_f8 = wpool.tile([P, 2, 16], f8, tag="ones_f8")
        nc.vector.memset(ones_f8, 1.0)

        # per-chunk persistent activations (separate tiles -> no false deps)
        kro_t = [qkv.tile([P, CH], bf16, tag=f"kro{ci}", name=f"kro{ci}")
                 for ci in range(NCH)]
        v_t = [qkv.tile([P, ST, P], bf16, tag=f"v{ci}", name=f"v{ci}")
               for ci in range(NCH)]
        qro_t = [None] * NCH
        eg_t = [None] * NCH

        def rope_head(ps, dst, cs_i):
            # all on the vector engine so interleaving with attention keeps
            # the scalar queue pure exp
            qf = work.tile([P, CH], f32, tag="qf")
            nc.vector.tensor_copy(out=qf, in_=ps)
            rot = work.tile([P, CH], f32, tag="rot")
            nc.vector.stream_shuffle(rot[0:64, :], qf[64:128, :], IDENT32)
            nc.vector.stream_shuffle(rot[64:128, :], qf[0:64, :], IDENT32)
            t1 = work.tile([P, CH], f32, tag="t1")
            nc.vector.tensor_mul(t1, qf, cos_sb[:, cs_i])
            t2 = work.tile([P, CH], f32, tag="t2")
            nc.vector.tensor_mul(t2, rot, sin_sb[:, cs_i])
            nc.vector.tensor_add(dst, t1, t2)

        def proj_fillers(ci):
            """PE work groups for chunk ci's projections."""
            xc = xcs[ci]
            cs_i = slice(ci * CH, (ci + 1) * CH)
            fs = []

            def k_g():
                psk = ps_mm.tile([P, CH], f32, tag="proj", name=f"k{ci}")
                for kt in range(KT):
                    nc.tensor.matmul(
                        psk, wk_sb[:, kt, :], xc[:, kt, :],
                        start=(kt == 0), stop=(kt == KT - 1),
                    )
                rope_head(psk, kro_t[ci][:, :], cs_i)
            fs.append(k_g)

            def v_g():
                psv = ps_mm.tile([P, CH], f32, tag="proj", name=f"v{ci}")
                for st in range(ST):
                    for kt in range(KT):
                        nc.tensor.matmul(
                            psv[:, st * P:(st + 1) * P],
                            xc[:, kt, st * P:(st + 1) * P], wv_sb[:, kt, :],
                            start=(kt == 0), stop=(kt == KT - 1),
                        )
                for st in range(ST):
                    nc.vector.tensor_copy(
                        out=v_t[ci][:, st, :], in_=psv[:, st * P:(st + 1) * P]
                    )
            fs.append(v_g)

            def q_g(qh):
                if qro_t[ci] is None:
                    qro_t[ci] = qrop.tile([P, QH, CH], bf16, tag="qro",
                                          name=f"qro{ci}")
                psq = ps_mm.tile([P, CH], f32, tag="proj", name=f"q{ci}_{qh}")
                for kt in range(KT):
                    nc.tensor.matmul(
                        psq, wq_sb[:, kt, qh * P:(qh + 1) * P], xc[:, kt, :],
                        start=(kt == 0), stop=(kt == KT - 1),
                    )
                rope_head(psq, qro_t[ci][:, qh, :], cs_i)
            for qh in range(QH):
                fs.append(lambda qh=qh: q_g(qh))

            def g_g(qh):
                if eg_t[ci] is None:
                    eg_t[ci] = egp.tile([P, QH, CH], bf16, tag="eg",
                                        name=f"eg{ci}")
                psg = ps_mm.tile([P, CH], f32, tag="proj", name=f"g{ci}_{qh}")
                for kt in range(KT):
                    nc.tensor.matmul(
                        psg, wg_sb[:, kt, qh * P:(qh + 1) * P], xc[:, kt, :],
                        start=(kt == 0), stop=(kt == KT - 1),
                    )
                nc.scalar.activation(
                    out=eg_t[ci][:, qh, :], in_=psg, func=expf,
                    bias=bgn_sb[:, qh:qh + 1], scale=-1.0,
                )
            for qh in range(QH):
                fs.append(lambda qh=qh: g_g(qh))

            if ci + 1 < NCH:
                def x_dma():
                    xcs[ci + 1] = xpool.tile([P, KT, CH], bf16, tag="xc",
                                             name=f"xc{ci + 1}")
                    nc.sync.dma_start(out=xcs[ci + 1], in_=xT[ci + 1, :, :, :])
                fs.append(x_dma)
            return fs

        def oproj_fillers(ci):
            """PE work groups for chunk ci's partial o_proj."""
            fs = []

            def st_g(st):
                r0 = ci * CH + st * P
                obt = outp.tile([P, HIDDEN], f32, tag="obt")
                for hp in range(2):
                    pss = [
                        ps_mm.tile([P, CH], f32, tag="proj", name=f"o{hi}")
                        for hi in range(2)
                    ]
                    for dt in range(QH):
                        for hi in range(2):
                            h0 = hp * 2 + hi
                            nc.tensor.matmul(
                                pss[hi],
                                ags[ci][:, dt, st * P:(st + 1) * P],
                                wo_sb[:, dt, h0 * CH:(h0 + 1) * CH],
                                start=(dt == 0), stop=(dt == QH - 1),
                            )
                    for hi in range(2):
                        h0 = hp * 2 + hi
                        nc.vector.tensor_copy(
                            out=obt[:, h0 * CH:(h0 + 1) * CH], in_=pss[hi]
                        )
                nc.sync.dma_start(out=out[r0:r0 + P, :], in_=obt)
            for st in range(ST):
                fs.append(lambda st=st: st_g(st))
            return fs

        # normalization fused with the sigmoid gate:
        # ag = at / ((1 + Eg) * denom)
        def norm_stage2(qh, at, dnsb, eg, ag):
            bc = ps_mm.tile([P, CH], f32, tag="proj", name="bc")
            nc.tensor.matmul(bc, ones_bc, dnsb, start=True, stop=True)
            w = nrm.tile([P, CH], f32, tag="w")
            nc.vector.scalar_tensor_tensor(
                out=w, in0=eg[:, qh, :], scalar=1.0, in1=bc,
                op0=mybir.AluOpType.add, op1=mybir.AluOpType.mult,
            )
            r = nrm.tile([P, CH], f32, tag="r")
            nc.vector.reciprocal_approx_fast(out=r, in_=w)
            nc.vector.tensor_mul(ag[:, qh, :], at, r)

        ags = [None] * NCH
        pending_norm = None

        # chunk 0 projections emitted directly
        for f in proj_fillers(0):
            f()

        for c in range(NCH):
            # filler queue drained at attention head boundaries: previous
            # chunk's o_proj first, then next chunk's projections
            fillers = []
            if c > 0:
                fillers += oproj_fillers(c - 1)
            if c + 1 < NCH:
                fillers += proj_fillers(c + 1)

            # ---- attention for this sq chunk: one flat (qh, t) stream ----
            ag = agp.tile([P, QH, CH], bf16, tag="ag")
            ags[c] = ag
            eg = eg_t[c]
            qro = qro_t[c]
            ntiles = (c + 1) * ST
            items = [(qh, t) for qh in range(QH) for t in range(ntiles)]
            sc_slots = {}

            def issue_sc(qh, t, c=c, qro=qro):
                o = t - c * ST
                lo = o * P if o > 0 else 0
                s = ps_sc.tile([P, CH], f32, tag="sc")
                nc.tensor.matmul(
                    s[:, lo:], kro_t[t // ST][:, (t % ST) * P:(t % ST + 1) * P],
                    qro[:, qh, lo:],
                    start=True, stop=True,
                )
                sc_slots[(qh, t)] = (s, lo)

            issue_sc(*items[0])
            if len(items) > 1:
                issue_sc(*items[1])
            at = dn = pr8 = None
            for i, (qh, t) in enumerate(items):
                if t == 0:
                    at = ps_at.tile([P, CH], f32, tag="attn")
                    dn = ps_dn.tile([1, CH], f32, tag="denom")
                s, lo = sc_slots.pop((qh, t))
                o = t - c * ST
                if o < 0:
                    # full tile: fp8 probs in pairs, DoubleRow denominator
                    j = t % 2
                    if j == 0:
                        pr8 = prp.tile([P, 2, CH], f8, tag="pr8")
                    nc.scalar.activation(
                        out=pr8[:, j, :], in_=s, func=expf, scale=SCALE,
                    )
                    nc.tensor.matmul(
                        at, v_t[t // ST][:, t % ST, :], pr8[:, j, :],
                        start=(t == 0), stop=False,
                    )
                    if j == 1:
                        nc.tensor.matmul(
                            dn, ones_f8[:, :, 0:1], pr8,
                            start=(t == 1), stop=False,
                            perf_mode=DROW,
                        )
                else:
                    pr = prp.tile([P, CH], bf16, tag="pr")
                    nc.scalar.activation(
                        out=pr[:, lo:], in_=s[:, lo:], func=expf, scale=SCALE,
                    )
                    nc.vector.tensor_mul(
                        pr[:, o * P:(o + 1) * P],
                        pr[:, o * P:(o + 1) * P], msk_sb,
                    )
                    nc.tensor.matmul(
                        at[:, lo:], v_t[t // ST][:, t % ST, :], pr[:, lo:],
                        start=(t == 0), stop=(t == ntiles - 1),
                    )
                    nc.tensor.matmul(
                        dn[:, lo:], ones_pv, pr[:, lo:],
                        start=(c == 0 and t == 0), stop=(t == ntiles - 1),
                    )
                if i + 2 < len(items):
                    issue_sc(*items[i + 2])
                if t == 1 and pending_norm is not None:
                    norm_stage2(*pending_norm)
                    pending_norm = None
                if t == ntiles - 1:
                    # stage 1 on the vector engine: frees the single dn bank
                    # without wedging a copy into the scalar exp stream
                    dnsb = nrm.tile([1, CH], bf16, tag="dnsb")
                    nc.vector.tensor_copy(out=dnsb, in_=dn)
                    pending_norm = (qh, at, dnsb, eg, ag)
                    if qh < QH - 1 and fillers:
                        fillers.pop(0)()
            # remaining fillers after the attention stream
            for f in fillers:
                f()

        # final chunk tail: last head's norm, then its o_proj
        norm_stage2(*pending_norm)
        for f in oproj_fillers(NCH - 1):
            f()

    nc.finalize()
    return nc


_PROGRAMS = {}


def _get_program(S=S_FULL):
    if S not in _PROGRAMS:
        _PROGRAMS[S] = build_program(S)
    return _PROGRAMS[S]


def _host_tables(position_ids_b, S):
    pos = np.asarray(position_ids_b, dtype=np.float32)  # [S]
    inv = 1.0 / (ROPE_THETA ** (np.arange(0, P, 2, dtype=np.float32) / P))  # [64]
    ang = np.concatenate([inv, inv]).astype(np.float32)[:, None] * pos[None, :]
    cosT = np.cos(ang).astype(np.float32)
    sgn = np.where(np.arange(P) < 64, -1.0, 1.0).astype(np.float32)
    sinT = (np.sin(ang) * sgn[:, None]).astype(np.float32)
    return cosT, sinT


def make_in_maps(x, position_ids, Wq, Wk, Wv, Wo, Wg, bg, S=S_FULL):
    NCH = S // CH
    x = np.asarray(x, dtype=np.float32)
    msk = (np.arange(P)[:, None] <= np.arange(P)[None, :]).astype(BF16)
    xT_b = []
    for b in range(B):
        xt = np.ascontiguousarray(x[b, :S].T)                    # [H, S]
        xt = xt.reshape(KT, P, NCH, CH).transpose(2, 1, 0, 3)    # [NCH,P,KT,CH]
        xT_b.append(np.ascontiguousarray(xt).astype(BF16))
    tabs = [_host_tables(np.asarray(position_ids)[b, :S], S) for b in range(B)]
    Wq = np.asarray(Wq, np.float32)
    Wk = np.asarray(Wk, np.float32)
    Wv = np.asarray(Wv, np.float32)
    Wo = np.asarray(Wo, np.float32)
    Wg = np.asarray(Wg, np.float32)
    bg = np.asarray(bg, np.float32)

    def warr(w):  # [H, N] -> [P, KT, N]
        n = w.shape[1]
        return np.ascontiguousarray(
            w.reshape(KT, P, n).transpose(1, 0, 2)).astype(BF16)

    maps = []
    for core in range(8):
        b, g = core // 4, core % 4
        cosT, sinT = tabs[b]
        wo_c = Wo[g * DQ:(g + 1) * DQ, :].reshape(QH, P, HIDDEN).transpose(1, 0, 2)
        bgn_c = (-bg[g * DQ:(g + 1) * DQ]).reshape(QH, P).T
        maps.append({
            "xT": xT_b[b],
            "wq": warr(Wq[:, g * DQ:(g + 1) * DQ]),
            "wk": warr(Wk[:, g * P:(g + 1) * P]),
            "wv": warr(Wv[:, g * P:(g + 1) * P]),
            "wg": warr(Wg[:, g * DQ:(g + 1) * DQ]),
            "wo": np.ascontiguousarray(wo_c).astype(BF16),
            "bgn": np.ascontiguousarray(bgn_c),
            "cosT": cosT,
            "sinT": sinT,
            "msk": msk,
        })
    return maps


def run(inputs, S=S_FULL, trace=False, **kw):
    nc = _get_program(S)
    maps = make_in_maps(S=S, **inputs)
    res = run_bass_kernel_spmd(nc, maps, core_ids=list(range(8)), trace=trace, **kw)
    out = np.zeros((B, S, HIDDEN), np.float32)
    for core in range(8):
        out[core // 4] += np.asarray(res.results[core]["out"], np.float32)
    return out, res


def kernel(x, position_ids, Wq, Wk, Wv, Wo, Wg, bg):
    out, _ = run(dict(x=x, position_ids=position_ids, Wq=Wq, Wk=Wk, Wv=Wv,
                      Wo=Wo, Wg=Wg, bg=bg))
    return out


# revision 20
# speedup vs baseline: 1.4288x; 1.0805x over previous
"""Trainium2 Bass kernel for LuluAttention (gated GQA attention + RoPE).

Sharding over 8 NeuronCores: core = b*4 + g where b = batch (2), g = head
group (4). Each core computes 4 Q heads + their shared KV head for one batch
element, plus the matching gate slice, and a partial o_proj output
(contraction over its 512 attn dims). Host sums the 4 partials per batch.

On-chip layouts are transposed ([dim, seq]) so the attention pipeline needs
no transposes:
  qT/kT [d, s] -> scoresT[sk, sq] = kT_tile.T @ qT_chunk
  softmax denominator via ones-matmul (partition reduction); full tiles use
  fp8 probs in pairs so the denominator runs as a DoubleRow matmul at
  0.5 cyc/row.  The denominator broadcast is a K=1 bf16 matmul; the
  reciprocal (reciprocal_approx_fast) is fused with the sigmoid gate:
    ag = at / ((1 + exp(-z_gate)) * denom)
  v kept straight [s, d] -> attnT[d, sq] = v_tile.T @ probsT.

Perf structure:
  - All DRAM tensors host-pre-arranged into SBUF layouts (contiguous
    per-partition DMA descriptors).
  - Causal narrowing on diagonal tiles + single [128,128] triangle mask.
  - One flat (head, tile) attention stream per chunk; scores issued two
    tiles ahead across head boundaries.
  - Filler queue: the NEXT chunk's projections and the PREVIOUS chunk's
    o_proj are emitted at attention head boundaries, so the PE fills the
    exp-latency slack and never idles; attention keeps the scalar engine's
    queue pure exp (all PSUM-drain copies run on the vector engine).
  - Per-chunk activation tiles (kro/qro/v/eg) avoid false write-read
    dependencies between interleaved chunks.
"""

import numpy as np
import ml_dtypes
from contextlib import ExitStack

import concourse.bass as bass
import concourse.bacc as bacc
import concourse.tile as tile
from concourse import mybir
from concourse.bass_utils import run_bass_kernel_spmd

BF16 = ml_dtypes.bfloat16
F8 = ml_dtypes.float8_e4m3fn

HIDDEN = 2048
B = 2
S_FULL = 2048
P = 128
CH = 512               # seq chunk width
QH = 4                 # q heads per core
DQ = QH * P            # 512 q dims per core
KT = HIDDEN // P       # 16 contraction tiles
SCALE = 1.0 / float(np.sqrt(128.0))
ROPE_THETA = 10000.0

IDENT32 = list(range(32))


def build_program(S=S_FULL):
    f32 = mybir.dt.float32
    bf16 = mybir.dt.bfloat16
    f8 = mybir.dt.float8e4
    expf = mybir.ActivationFunctionType.Exp
    DROW = mybir.MatmulPerfMode.DoubleRow

    NCH = S // CH
    ST = CH // P           # 4 seq sub-tiles per chunk

    nc = bacc.Bacc("TRN2", debug=False, target_bir_lowering=False)

    xT = nc.declare_dram_parameter("xT", [NCH, P, KT, CH], bf16, False)
    wq = nc.declare_dram_parameter("wq", [P, KT, DQ], bf16, False)
    wk = nc.declare_dram_parameter("wk", [P, KT, P], bf16, False)
    wv = nc.declare_dram_parameter("wv", [P, KT, P], bf16, False)
    wg = nc.declare_dram_parameter("wg", [P, KT, DQ], f8, False)
    xg8 = nc.declare_dram_parameter("xg8", [NCH, P, KT, CH], f8, False)
    wo = nc.declare_dram_parameter("wo", [P, QH, HIDDEN], bf16, False)
    bgn = nc.declare_dram_parameter("bgn", [P, QH], f32, False)
    cosT = nc.declare_dram_parameter("cosT", [P, S], f32, False)
    sinT = nc.declare_dram_parameter("sinT", [P, S], f32, False)
    msk = nc.declare_dram_parameter("msk", [P, P], bf16, False)
    out = nc.declare_dram_parameter("out", [S, HIDDEN], f32, True)

    with tile.TileContext(nc) as tc, ExitStack() as ctx:
        wpool = ctx.enter_context(tc.tile_pool(name="weights", bufs=1))
        xpool = ctx.enter_context(tc.tile_pool(name="xchunks", bufs=2))
        qkv = ctx.enter_context(tc.tile_pool(name="qkv", bufs=1))
        qrop = ctx.enter_context(tc.tile_pool(name="qrop", bufs=2))
        egp = ctx.enter_context(tc.tile_pool(name="eg", bufs=2))
        work = ctx.enter_context(tc.tile_pool(name="work", bufs=2))
        prp = ctx.enter_context(tc.tile_pool(name="probs", bufs=4))
        nrm = ctx.enter_context(tc.tile_pool(name="nrm", bufs=2))
        agp = ctx.enter_context(tc.tile_pool(name="agp", bufs=2))
        outp = ctx.enter_context(tc.tile_pool(name="outp", bufs=2))
        ps_mm = ctx.enter_context(tc.tile_pool(name="ps_mm", bufs=2, space="PSUM"))
        ps_sc = ctx.enter_context(tc.tile_pool(name="ps_sc", bufs=2, space="PSUM"))
        ps_at = ctx.enter_context(tc.tile_pool(name="ps_at", bufs=2, space="PSUM"))
        ps_dn = ctx.enter_context(tc.tile_pool(name="ps_dn", bufs=1, space="PSUM"))

        # ---- persistent loads; x chunk 0 first so k/v proj start ASAP.
        # Early loads split into several dma_starts to spread across queues.
        xcs = [None] * NCH
        xcs[0] = xpool.tile([P, KT, CH], bf16, tag="xc", name="xc0")
        for k0 in range(0, KT, 4):
            nc.sync.dma_start(out=xcs[0][:, k0:k0 + 4, :],
                              in_=xT[0, :, k0:k0 + 4, :])
        wk_sb = wpool.tile([P, KT, P], bf16, tag="wk")
        for k0 in range(0, KT, 8):
            nc.sync.dma_start(out=wk_sb[:, k0:k0 + 8, :],
                              in_=wk[:, k0:k0 + 8, :])
        wv_sb = wpool.tile([P, KT, P], bf16, tag="wv")
        nc.sync.dma_start(out=wv_sb, in_=wv[:, :, :])
        cos_sb = wpool.tile([P, S], f32, tag="cos")
        nc.sync.dma_start(out=cos_sb, in_=cosT[:, :])
        sin_sb = wpool.tile([P, S], f32, tag="sin")
        nc.sync.dma_start(out=sin_sb, in_=sinT[:, :])
        wq_sb = wpool.tile([P, KT, DQ], bf16, tag="wq")
        for k0 in range(0, KT, 8):
            nc.sync.dma_start(out=wq_sb[:, k0:k0 + 8, :],
                              in_=wq[:, k0:k0 + 8, :])
        msk_sb = wpool.tile([P, P], bf16, tag="msk")
        nc.sync.dma_start(out=msk_sb, in_=msk[:, :])
        wg_sb = wpool.tile([P, KT, DQ], f8, tag="wg")
        nc.sync.dma_start(out=wg_sb, in_=wg[:, :, :])
        xg8s = [None] * NCH
        xg8s[0] = xpool.tile([P, KT, CH], f8, tag="xg8", name="xg80")
        nc.sync.dma_start(out=xg8s[0], in_=xg8[0, :, :, :])
        bgn_sb = wpool.tile([P, QH], f32, tag="bgn")
        nc.sync.dma_start(out=bgn_sb, in_=bgn[:, :])
        wo_sb = wpool.tile([P, QH, HIDDEN], bf16, tag="wo")
        nc.sync.dma_start(out=wo_sb, in_=wo[:, :, :])
        ones_pv = wpool.tile([P, 1], bf16, tag="ones_pv")
        nc.vector.memset(ones_pv, 1.0)
        ones_bc = wpool.tile([1, P], bf16, tag="ones_bc")
        nc.vector.memset(ones_bc, 1.0)
        # fp8 ones for the DoubleRow denominator; 2-dim stride kept 16B-aligned
        ones_f8 = wpool.tile([P, 2, 16], f8, tag="ones_f8")
        nc.vector.memset(ones_f8, 1.0)

        # per-chunk persistent activations (separate tiles -> no false deps)
        kro_t = [qkv.tile([P, CH], bf16, tag=f"kro{ci}", name=f"kro{ci}")
                 for ci in range(NCH)]
        v_t = [qkv.tile([P, ST, P], bf16, tag=f"v{ci}", name=f"v{ci}")
               for ci in range(NCH)]
        qro_t = [None] * NCH
        eg_t = [None] * NCH

        def rope_head(ps, dst, cs_i):
            # all on the vector engine so interleaving with attention keeps
            # the scalar queue pure exp
            qf = work.tile([P, CH], f32, tag="qf")
            nc.vector.tensor_copy(out=qf, in_=ps)
            rot = work.tile([P, CH], f32, tag="rot")
            nc.vector.stream_shuffle(rot[0:64, :], qf[64:128, :], IDENT32)
            nc.vector.stream_shuffle(rot[64:128, :], qf[0:64, :], IDENT32)
            t1 = work.tile([P, CH], f32, tag="t1")
            nc.vector.tensor_mul(t1, qf, cos_sb[:, cs_i])
            t2 = work.tile([P, CH], f32, tag="t2")
            nc.vector.tensor_mul(t2, rot, sin_sb[:, cs_i])
            nc.vector.tensor_add(dst, t1, t2)

        def proj_fillers(ci):
            """PE work groups for chunk ci's projections."""
            xc = xcs[ci]
            cs_i = slice(ci * CH, (ci + 1) * CH)
            fs = []

            def k_g():
                psk = ps_mm.tile([P, CH], f32, tag="proj", name=f"k{ci}")
                for kt in range(KT):
                    nc.tensor.matmul(
                        psk, wk_sb[:, kt, :], xc[:, kt, :],
                        start=(kt == 0), stop=(kt == KT - 1),
                    )
                rope_head(psk, kro_t[ci][:, :], cs_i)
            fs.append(k_g)

            def v_g():
                psv = ps_mm.tile([P, CH], f32, tag="proj", name=f"v{ci}")
                for st in range(ST):
                    for kt in range(KT):
                        nc.tensor.matmul(
                            psv[:, st * P:(st + 1) * P],
                            xc[:, kt, st * P:(st + 1) * P], wv_sb[:, kt, :],
                            start=(kt == 0), stop=(kt == KT - 1),
                        )
                for st in range(ST):
                    nc.vector.tensor_copy(
                        out=v_t[ci][:, st, :], in_=psv[:, st * P:(st + 1) * P]
                    )
            fs.append(v_g)

            def q_g(qh):
                if qro_t[ci] is None:
                    qro_t[ci] = qrop.tile([P, QH, CH], bf16, tag="qro",
                                          name=f"qro{ci}")
                psq = ps_mm.tile([P, CH], f32, tag="proj", name=f"q{ci}_{qh}")
                for kt in range(KT):
                    nc.tensor.matmul(
                        psq, wq_sb[:, kt, qh * P:(qh + 1) * P], xc[:, kt, :],
                        start=(kt == 0), stop=(kt == KT - 1),
                    )
                rope_head(psq, qro_t[ci][:, qh, :], cs_i)
            for qh in range(QH):
                fs.append(lambda qh=qh: q_g(qh))

            def g_g(qh):
                if eg_t[ci] is None:
                    eg_t[ci] = egp.tile([P, QH, CH], bf16, tag="eg",
                                        name=f"eg{ci}")
                psg = ps_mm.tile([P, CH], f32, tag="proj", name=f"g{ci}_{qh}")
                xg = xg8s[ci]
                for k2 in range(KT // 2):
                    nc.tensor.matmul(
                        psg,
                        wg_sb[:, 2 * k2:2 * k2 + 2, qh * P:(qh + 1) * P],
                        xg[:, 2 * k2:2 * k2 + 2, :],
                        start=(k2 == 0), stop=(k2 == KT // 2 - 1),
                        perf_mode=DROW,
                    )
                # Wg was pre-scaled by 64 on the host for the fp8 range
                nc.scalar.activation(
                    out=eg_t[ci][:, qh, :], in_=psg, func=expf,
                    bias=bgn_sb[:, qh:qh + 1], scale=-1.0 / 64.0,
                )
            for qh in range(QH):
                fs.append(lambda qh=qh: g_g(qh))

            if ci + 1 < NCH:
                def x_dma():
                    xcs[ci + 1] = xpool.tile([P, KT, CH], bf16, tag="xc",
                                             name=f"xc{ci + 1}")
                    nc.sync.dma_start(out=xcs[ci + 1], in_=xT[ci + 1, :, :, :])
                    xg8s[ci + 1] = xpool.tile([P, KT, CH], f8, tag="xg8",
                                              name=f"xg8{ci + 1}")
                    nc.sync.dma_start(out=xg8s[ci + 1],
                                      in_=xg8[ci + 1, :, :, :])
                fs.append(x_dma)
            return fs

        def oproj_fillers(ci):
            """PE work groups for chunk ci's partial o_proj."""
            fs = []

            def st_g(st):
                r0 = ci * CH + st * P
                obt = outp.tile([P, HIDDEN], f32, tag="obt")
                for hp in range(2):
                    pss = [
                        ps_mm.tile([P, CH], f32, tag="proj", name=f"o{hi}")
                        for hi in range(2)
                    ]
                    for dt in range(QH):
                        for hi in range(2):
                            h0 = hp * 2 + hi
                            nc.tensor.matmul(
                                pss[hi],
                                ags[ci][:, dt, st * P:(st + 1) * P],
                                wo_sb[:, dt, h0 * CH:(h0 + 1) * CH],
                                start=(dt == 0), stop=(dt == QH - 1),
                            )
                    for hi in range(2):
                        h0 = hp * 2 + hi
                        nc.vector.tensor_copy(
                            out=obt[:, h0 * CH:(h0 + 1) * CH], in_=pss[hi]
                        )
                nc.sync.dma_start(out=out[r0:r0 + P, :], in_=obt)
            for st in range(ST):
                fs.append(lambda st=st: st_g(st))
            return fs

        # normalization fused with the sigmoid gate:
        # ag = at / ((1 + Eg) * denom)
        def norm_stage2(qh, at, dnsb, eg, ag):
            bc = ps_mm.tile([P, CH], f32, tag="proj", name="bc")
            nc.tensor.matmul(bc, ones_bc, dnsb, start=True, stop=True)
            w = nrm.tile([P, CH], f32, tag="w")
            nc.vector.scalar_tensor_tensor(
                out=w, in0=eg[:, qh, :], scalar=1.0, in1=bc,
                op0=mybir.AluOpType.add, op1=mybir.AluOpType.mult,
            )
            r = nrm.tile([P, CH], f32, tag="r")
            nc.vector.reciprocal_approx_fast(out=r, in_=w)
            nc.vector.tensor_mul(ag[:, qh, :], at, r)

        ags = [None] * NCH
        pending_norm = None

        # chunk 0 projections emitted directly
        for f in proj_fillers(0):
            f()

        for c in range(NCH):
            # filler queue drained at attention head boundaries: previous
            # chunk's o_proj first, then next chunk's projections
            fillers = []
            if c > 0:
                fillers += oproj_fillers(c - 1)
            if c + 1 < NCH:
                fillers += proj_fillers(c + 1)

            # ---- attention for this sq chunk: one flat (qh, t) stream ----
            ag = agp.tile([P, QH, CH], bf16, tag="ag")
            ags[c] = ag
            eg = eg_t[c]
            qro = qro_t[c]
            ntiles = (c + 1) * ST
            items = [(qh, t) for qh in range(QH) for t in range(ntiles)]
            sc_slots = {}

            def issue_sc(qh, t, c=c, qro=qro):
                o = t - c * ST
                lo = o * P if o > 0 else 0
                s = ps_sc.tile([P, CH], f32, tag="sc")
                nc.tensor.matmul(
                    s[:, lo:], kro_t[t // ST][:, (t % ST) * P:(t % ST + 1) * P],
                    qro[:, qh, lo:],
                    start=True, stop=True,
                )
                sc_slots[(qh, t)] = (s, lo)

            issue_sc(*items[0])
            if len(items) > 1:
                issue_sc(*items[1])
            at = dn = pr8 = None
            for i, (qh, t) in enumerate(items):
                if t == 0:
                    at = ps_at.tile([P, CH], f32, tag="attn")
                    dn = ps_dn.tile([1, CH], f32, tag="denom")
                s, lo = sc_slots.pop((qh, t))
                o = t - c * ST
                if o < 0:
                    # full tile: fp8 probs in pairs, DoubleRow denominator
                    j = t % 2
                    if j == 0:
                        pr8 = prp.tile([P, 2, CH], f8, tag="pr8")
                    nc.scalar.activation(
                        out=pr8[:, j, :], in_=s, func=expf, scale=SCALE,
                    )
                    nc.tensor.matmul(
                        at, v_t[t // ST][:, t % ST, :], pr8[:, j, :],
                        start=(t == 0), stop=False,
                    )
                    if j == 1:
                        nc.tensor.matmul(
                            dn, ones_f8[:, :, 0:1], pr8,
                            start=(t == 1), stop=False,
                            perf_mode=DROW,
                        )
                else:
                    pr = prp.tile([P, CH], bf16, tag="pr")
                    nc.scalar.activation(
                        out=pr[:, lo:], in_=s[:, lo:], func=expf, scale=SCALE,
                    )
                    nc.vector.tensor_mul(
                        pr[:, o * P:(o + 1) * P],
                        pr[:, o * P:(o + 1) * P], msk_sb,
                    )
                    nc.tensor.matmul(
                        at[:, lo:], v_t[t // ST][:, t % ST, :], pr[:, lo:],
                        start=(t == 0), stop=(t == ntiles - 1),
                    )
                    nc.tensor.matmul(
                        dn[:, lo:], ones_pv, pr[:, lo:],
                        start=(c == 0 and t == 0), stop=(t == ntiles - 1),
                    )
                if i + 2 < len(items):
                    issue_sc(*items[i + 2])
                if t == 1 and pending_norm is not None:
                    norm_stage2(*pending_norm)
                    pending_norm = None
                if t == ntiles - 1:
                    # stage 1 on the vector engine: frees the single dn bank
                    # without wedging a copy into the scalar exp stream
                    dnsb = nrm.tile([1, CH], bf16, tag="dnsb")
                    nc.vector.tensor_copy(out=dnsb, in_=dn)
                    pending_norm = (qh, at, dnsb, eg, ag)
                    if qh < QH - 1 and fillers:
                        fillers.pop(0)()
            # remaining fillers after the attention stream
            for f in fillers:
                f()

        # final chunk tail: last head's norm, then its o_proj
        norm_stage2(*pending_norm)
        for f in oproj_fillers(NCH - 1):
            f()

    nc.finalize()
    return nc


_PROGRAMS = {}


def _get_program(S=S_FULL):
    if S not in _PROGRAMS:
        _PROGRAMS[S] = build_program(S)
    return _PROGRAMS[S]


def _host_tables(position_ids_b, S):
    pos = np.asarray(position_ids_b, dtype=np.float32)  # [S]
    inv = 1.0 / (ROPE_THETA ** (np.arange(0, P, 2, dtype=np.float32) / P))  # [64]
    ang = np.concatenate([inv, inv]).astype(np.float32)[:, None] * pos[None, :]
    cosT = np.cos(ang).astype(np.float32)
    sgn = np.where(np.arange(P) < 64, -1.0, 1.0).astype(np.float32)
    sinT = (np.sin(ang) * sgn[:, None]).astype(np.float32)
    return cosT, sinT


def make_in_maps(x, position_ids, Wq, Wk, Wv, Wo, Wg, bg, S=S_FULL):
    NCH = S // CH
    x = np.asarray(x, dtype=np.float32)
    msk = (np.arange(P)[:, None] <= np.arange(P)[None, :]).astype(BF16)
    xT_b = []
    xg8_b = []
    for b in range(B):
        xt = np.ascontiguousarray(x[b, :S].T)                    # [H, S]
        xt = xt.reshape(KT, P, NCH, CH).transpose(2, 1, 0, 3)    # [NCH,P,KT,CH]
        xt = np.ascontiguousarray(xt)
        xT_b.append(xt.astype(BF16))
        xg8_b.append(xt.astype(F8))
    tabs = [_host_tables(np.asarray(position_ids)[b, :S], S) for b in range(B)]
    Wq = np.asarray(Wq, np.float32)
    Wk = np.asarray(Wk, np.float32)
    Wv = np.asarray(Wv, np.float32)
    Wo = np.asarray(Wo, np.float32)
    Wg = np.asarray(Wg, np.float32)
    bg = np.asarray(bg, np.float32)

    def warr(w):  # [H, N] -> [P, KT, N]
        n = w.shape[1]
        return np.ascontiguousarray(
            w.reshape(KT, P, n).transpose(1, 0, 2)).astype(BF16)

    maps = []
    for core in range(8):
        b, g = core // 4, core % 4
        cosT, sinT = tabs[b]
        wo_c = Wo[g * DQ:(g + 1) * DQ, :].reshape(QH, P, HIDDEN).transpose(1, 0, 2)
        bgn_c = (-bg[g * DQ:(g + 1) * DQ]).reshape(QH, P).T
        maps.append({
            "xT": xT_b[b],
            "wq": warr(Wq[:, g * DQ:(g + 1) * DQ]),
            "wk": warr(Wk[:, g * P:(g + 1) * P]),
            "wv": warr(Wv[:, g * P:(g + 1) * P]),
            "wg": warr(Wg[:, g * DQ:(g + 1) * DQ] * 64.0).astype(F8),
            "xg8": xg8_b[b],
            "wo": np.ascontiguousarray(wo_c).astype(BF16),
            "bgn": np.ascontiguousarray(bgn_c),
            "cosT": cosT,
            "sinT": sinT,
            "msk": msk,
        })
    return maps


def run(inputs, S=S_FULL, trace=False, **kw):
    nc = _get_program(S)
    maps = make_in_maps(S=S, **inputs)
    res = run_bass_kernel_spmd(nc, maps, core_ids=list(range(8)), trace=trace, **kw)
    out = np.zeros((B, S, HIDDEN), np.float32)
    for core in range(8):
        out[core // 4] += np.asarray(res.results[core]["out"], np.float32)
    return out, res


def kernel(x, position_ids, Wq, Wk, Wv, Wo, Wg, bg):
    out, _ = run(dict(x=x, position_ids=position_ids, Wq=Wq, Wk=Wk, Wv=Wv,
                      Wo=Wo, Wg=Wg, bg=bg))
    return out
